# revision 3
# baseline (speedup 1.0000x reference)
"""Multi-head self-attention (B=4, N=2048, F=1024, 16 heads) on 8 TRN2 NeuronCores.

Sharding (Megatron-style, per the hint): data-parallel over the 4 batches x
tensor-parallel split of the 16 heads into 2 groups of 8. Core c handles
batch c//2 with head group c%2 (512 of the 1024 qkv features, column-split
Wq/Wk/Wv, row-split Wo). Each core emits a partial o-projection [2048, 1024];
the host unshard sums the pair of partials per batch (the Megatron
all-reduce) and stacks batches.

Device kernel layout choices (all matmuls bf16 with f32 PSUM accumulation):
  xT   [1024f, 2048i]  (x transposed on host)  - rhs for q/k, lhsT for v
  qT/kT [512o, 2048i]  (o = head-major features, on partitions)
  vAug [2048j, 8h, 65] (per head: V columns 0..63 plus a ones column at 64
                        so the attention-value matmul also yields the softmax
                        denominator Z as output row 64)
  scores S^T [j, i] via lhsT=kT-chunk, rhs=qT; exp on ScalarE (scale=1/32,
  no max subtraction needed: |S/32| <~ 1.5); attnU^T accumulated over j in
  PSUM; normalization by 1/Z via DVE with a DMA partition-broadcast of 1/Z.
"""

import sys
import types

sys.path.insert(0, "/opt/trn_rl_repo")

import numpy as np

# Best-effort: register the axon NTFF profile hook so trace=True works
# (used by test harnesses via BASS_TRACE); harmless when tracing is off.
try:
    import antenv

    if "antenv.axon_hooks" not in sys.modules:
        from trn_agent_boot.trn_boot import _ntff_profile_via_ctypes

        _hooks_mod = types.ModuleType("antenv.axon_hooks")
        _hook = _ntff_profile_via_ctypes("/opt/axon/libaxon_pjrt.so")
        _hooks_mod.get_axon_ntff_profile_hook = lambda: _hook
        _hooks_mod.set_axon_ntff_profile_hook = lambda h: None
        sys.modules["antenv.axon_hooks"] = _hooks_mod
        antenv.axon_hooks = _hooks_mod
except Exception:
    pass

import concourse.bacc as bacc
import concourse.tile as tile
from concourse import mybir
from concourse.bass_utils import run_bass_kernel_spmd

B, N, F = 4, 2048, 1024
HEAD, HD = 16, 64
NCORES = 8
HG = 2                # head groups (tensor-parallel degree per batch)
FL = F // HG          # local features per core = 512
HL = HEAD // HG       # local heads per core = 8
OC = FL // 128        # o-chunks of 128 in qT/kT = 4
FT = F // 128         # f (contraction) tiles = 8
IC = N // 128         # i-chunks of 128 = 16
ISPAN = 1024          # attention i-span per inner block
NSP = N // ISPAN      # spans = 2

BF16 = mybir.dt.bfloat16
F32 = mybir.dt.float32
NP_BF16 = mybir.dt.np(BF16)

_CACHED_NC = None
LAST_EXEC_TIME_NS = None


def _build_nc():
    nc = bacc.Bacc("TRN2")

    xT = nc.dram_tensor("xT", [F, N], BF16, kind="ExternalInput")
    wqT = nc.dram_tensor("wqT", [F, FL], BF16, kind="ExternalInput")
    wkT = nc.dram_tensor("wkT", [F, FL], BF16, kind="ExternalInput")
    wvT = nc.dram_tensor("wvT", [F, FL], BF16, kind="ExternalInput")
    woT = nc.dram_tensor("woT", [FL, F], BF16, kind="ExternalInput")
    bq = nc.dram_tensor("bq", [FL], F32, kind="ExternalInput")
    bk = nc.dram_tensor("bk", [FL], F32, kind="ExternalInput")
    bv = nc.dram_tensor("bv", [FL], F32, kind="ExternalInput")
    boh = nc.dram_tensor("boh", [F], F32, kind="ExternalInput")
    out = nc.dram_tensor("out", [N, F], F32, kind="ExternalOutput")

    with tile.TileContext(nc) as tc:
        with (
            tc.tile_pool(name="big", bufs=1) as big,
            tc.tile_pool(name="etile", bufs=3) as epool,
            tc.tile_pool(name="ztile", bufs=2) as zpool,
            tc.tile_pool(name="rbc", bufs=2) as rpool,
            tc.tile_pool(name="ostage", bufs=3) as opool,
            tc.tile_pool(name="dspill", bufs=2, space="DRAM") as dpool,
            tc.tile_pool(name="pp", bufs=2, space="PSUM") as pp,
            tc.tile_pool(name="sp", bufs=2, space="PSUM") as spp,
            tc.tile_pool(name="aup", bufs=1, space="PSUM") as aup,
        ):
            # ---- resident SBUF tensors ----
            xT_sb = big.tile([128, FT, N], BF16, tag="xT")
            wqT_sb = big.tile([128, FT, FL], BF16, tag="wqT")
            wkT_sb = big.tile([128, FT, FL], BF16, tag="wkT")
            wvT_sb = big.tile([128, FT, FL], BF16, tag="wvT")
            woT_sb = big.tile([128, OC, F], BF16, tag="woT")
            qT_sb = big.tile([128, OC, N], BF16, tag="qT")
            kT_sb = big.tile([128, OC, N], BF16, tag="kT")
            vAug_sb = big.tile([128, IC, HL, HD + 1], BF16, tag="vAug")
            attnT_sb = big.tile([128, OC, N], BF16, tag="attnT")
            bq_sb = big.tile([128, OC], F32, tag="bq")
            bk_sb = big.tile([128, OC], F32, tag="bk")
            bv_sb = big.tile([128, FL], F32, tag="bv")
            bo_sb = big.tile([128, F], F32, tag="bo")

            # ---- input DMAs ----
            nc.sync.dma_start(
                out=xT_sb[:], in_=xT.rearrange("(t p) i -> p t i", p=128)
            )
            nc.sync.dma_start(
                out=wqT_sb[:], in_=wqT.rearrange("(t p) o -> p t o", p=128)
            )
            nc.sync.dma_start(
                out=wkT_sb[:], in_=wkT.rearrange("(t p) o -> p t o", p=128)
            )
            nc.sync.dma_start(
                out=wvT_sb[:], in_=wvT.rearrange("(t p) o -> p t o", p=128)
            )
            nc.sync.dma_start(
                out=woT_sb[:], in_=woT.rearrange("(t p) g -> p t g", p=128)
            )
            nc.sync.dma_start(
                out=bq_sb[:], in_=bq.rearrange("(c p) -> p c", p=128)
            )
            nc.sync.dma_start(
                out=bk_sb[:], in_=bk.rearrange("(c p) -> p c", p=128)
            )
            nc.sync.dma_start(out=bv_sb[:], in_=bv[None, :].partition_broadcast(128))
            nc.sync.dma_start(out=bo_sb[:], in_=boh[None, :].partition_broadcast(128))
            # ones column (64) for every head; V overwrites columns 0..63
            nc.vector.memset(vAug_sb[:], 1.0)

            # ---- QKV projections ----
            # qT/kT: [o-chunk 128, i 512] = sum_f wT[f,o-chunk].T @ xT[f, i]
            for w_sb, b_sb, dst in ((wqT_sb, bq_sb, qT_sb), (wkT_sb, bk_sb, kT_sb)):
                for oc in range(OC):
                    for ic in range(N // 512):
                        ps = pp.tile([128, 512], F32, tag="pp")
                        for t in range(FT):
                            nc.tensor.matmul(
                                ps[:],
                                lhsT=w_sb[:, t, oc * 128 : (oc + 1) * 128],
                                rhs=xT_sb[:, t, ic * 512 : (ic + 1) * 512],
                                start=(t == 0),
                                stop=(t == FT - 1),
                            )
                        nc.vector.tensor_scalar_add(
                            out=dst[:, oc, ic * 512 : (ic + 1) * 512],
                            in0=ps[:],
                            scalar1=b_sb[:, oc : oc + 1],
                        )
            # v: [i-chunk 128, o 512] = sum_f xT[f, i-chunk].T @ wvT[f, o]
            for ic in range(IC):
                ps = pp.tile([128, 512], F32, tag="pp")
                for t in range(FT):
                    nc.tensor.matmul(
                        ps[:],
                        lhsT=xT_sb[:, t, ic * 128 : (ic + 1) * 128],
                        rhs=wvT_sb[:, t, :],
                        start=(t == 0),
                        stop=(t == FT - 1),
                    )
                nc.vector.tensor_add(
                    out=vAug_sb[:, ic, :, 0:HD],
                    in0=ps.rearrange("p (h d) -> p h d", h=HL),
                    in1=bv_sb.rearrange("p (h d) -> p h d", h=HL),
                )

            # ---- attention, head by head ----
            for h in range(HL):
                oc, sub = h // 2, h % 2
                p0 = sub * 64
                qh = qT_sb[p0 : p0 + 64, oc, :]
                kh = kT_sb[p0 : p0 + 64, oc, :]
                for isp in range(NSP):
                    i0 = isp * ISPAN
                    au = aup.tile([HD + 1, ISPAN], F32, tag="au")
                    for j in range(IC):
                        st = spp.tile([128, ISPAN], F32, tag="sp")
                        for ii in range(ISPAN // 512):
                            nc.tensor.matmul(
                                st[:, ii * 512 : (ii + 1) * 512],
                                lhsT=kh[:, j * 128 : (j + 1) * 128],
                                rhs=qh[:, i0 + ii * 512 : i0 + (ii + 1) * 512],
                                start=True,
                                stop=True,
                            )
                        eT = epool.tile([128, ISPAN], BF16, tag="eT")
                        nc.scalar.activation(
                            eT[:], st[:], mybir.ActivationFunctionType.Exp,
                            scale=1.0 / 32.0,
                        )
                        for ii in range(ISPAN // 512):
                            nc.tensor.matmul(
                                au[:, ii * 512 : (ii + 1) * 512],
                                lhsT=vAug_sb[:, j, h, :],
                                rhs=eT[:, ii * 512 : (ii + 1) * 512],
                                start=(j == 0),
                                stop=(j == IC - 1),
                            )
                    # softmax normalization: rows 0..63 are attnU, row 64 is Z
                    zr = zpool.tile([1, ISPAN], F32, tag="z")
                    nc.vector.reciprocal(zr[:], au[HD : HD + 1, :])
                    zt = dpool.tile([1, ISPAN], F32, tag="zspill")
                    nc.sync.dma_start(out=zt[:], in_=zr[:])
                    rb = rpool.tile([64, ISPAN], F32, tag="rb")
                    nc.sync.dma_start(out=rb[:], in_=zt[0, :].partition_broadcast(64))
                    nc.vector.tensor_mul(
                        out=attnT_sb[p0 : p0 + 64, oc, i0 : i0 + ISPAN],
                        in0=au[0:HD, :],
                        in1=rb[:],
                    )

            # ---- o projection (partial: local 512 features only) ----
            for ic in range(IC):
                for gc in range(F // 512):
                    ps = pp.tile([128, 512], F32, tag="pp")
                    for ct in range(OC):
                        nc.tensor.matmul(
                            ps[:],
                            lhsT=attnT_sb[:, ct, ic * 128 : (ic + 1) * 128],
                            rhs=woT_sb[:, ct, gc * 512 : (gc + 1) * 512],
                            start=(ct == 0),
                            stop=(ct == OC - 1),
                        )
                    st = opool.tile([128, 512], F32, tag="ost")
                    nc.vector.tensor_add(
                        out=st[:], in0=ps[:], in1=bo_sb[:, gc * 512 : (gc + 1) * 512]
                    )
                    nc.sync.dma_start(
                        out=out[ic * 128 : (ic + 1) * 128, gc * 512 : (gc + 1) * 512],
                        in_=st[:],
                    )

    nc.finalize()
    return nc


def kernel(x, Wq, bq, Wk, bk, Wv, bv, Wo, bo, trace=False):
    global _CACHED_NC, LAST_EXEC_TIME_NS
    x = np.asarray(x)
    Wq, Wk, Wv, Wo = (np.asarray(a) for a in (Wq, Wk, Wv, Wo))
    bq, bk, bv, bo = (np.asarray(a) for a in (bq, bk, bv, bo))

    if _CACHED_NC is None:
        _CACHED_NC = _build_nc()
    nc = _CACHED_NC

    # host-side shard prep (transposes + bf16 casts)
    xT_b = [np.ascontiguousarray(x[b].T).astype(NP_BF16) for b in range(B)]
    wqT_g = [np.ascontiguousarray(Wq[g * FL : (g + 1) * FL, :].T).astype(NP_BF16) for g in range(HG)]
    wkT_g = [np.ascontiguousarray(Wk[g * FL : (g + 1) * FL, :].T).astype(NP_BF16) for g in range(HG)]
    wvT_g = [np.ascontiguousarray(Wv[g * FL : (g + 1) * FL, :].T).astype(NP_BF16) for g in range(HG)]
    woT_g = [np.ascontiguousarray(Wo[:, g * FL : (g + 1) * FL].T).astype(NP_BF16) for g in range(HG)]
    bq_g = [np.ascontiguousarray(bq[g * FL : (g + 1) * FL]).astype(np.float32) for g in range(HG)]
    bk_g = [np.ascontiguousarray(bk[g * FL : (g + 1) * FL]).astype(np.float32) for g in range(HG)]
    bv_g = [np.ascontiguousarray(bv[g * FL : (g + 1) * FL]).astype(np.float32) for g in range(HG)]
    bo_half = (bo.astype(np.float32) / 2.0)

    in_maps = []
    for c in range(NCORES):
        b, g = c // HG, c % HG
        in_maps.append(
            {
                "xT": xT_b[b],
                "wqT": wqT_g[g],
                "wkT": wkT_g[g],
                "wvT": wvT_g[g],
                "woT": woT_g[g],
                "bq": bq_g[g],
                "bk": bk_g[g],
                "bv": bv_g[g],
                "boh": bo_half,
            }
        )

    res = run_bass_kernel_spmd(nc, in_maps, core_ids=list(range(NCORES)), trace=trace)
    LAST_EXEC_TIME_NS = res.exec_time_ns

    out = np.empty((B, N, F), np.float32)
    for b in range(B):
        out[b] = res.results[2 * b]["out"] + res.results[2 * b + 1]["out"]
    return out


# revision 6
# speedup vs baseline: 1.4906x; 1.4906x over previous
"""Multi-head self-attention (B=4, N=2048, F=1024, 16 heads) on 8 TRN2 NeuronCores.

Sharding (Megatron-style, per the hint): data-parallel over the 4 batches x
tensor-parallel split of the 16 heads into 2 groups of 8. Core c handles
batch c//2 with head group c%2 (512 of the 1024 qkv features, column-split
Wq/Wk/Wv, row-split Wo). Each core emits a partial o-projection [2048, 1024];
the host unshard sums the pair of partials per batch (the Megatron
all-reduce) and stacks batches.

Device kernel layout choices (all matmuls bf16 with f32 PSUM accumulation):
  xT   [1024f, 2048i]  (x transposed on host)  - rhs for q/k, lhsT for v
  qT/kT [512o, 2048i]  (o = head-major features, on partitions)
  vAug [2048j, 8h, 65] (per head: V columns 0..63 plus a ones column at 64
                        so the attention-value matmul also yields the softmax
                        denominator Z as output row 64)
  scores S^T [j, i] via lhsT=kT-chunk, rhs=qT; exp on ScalarE (scale=1/32,
  no max subtraction needed: |S/32| <~ 1.5); attnU^T accumulated over j in
  PSUM, copied to SBUF promptly to release the PSUM bank; softmax
  normalization (1/Z broadcast) runs off the critical path via small DMA
  round-trips through DRAM.

QKV is interleaved with attention per 128-row chunk of q/k so the ScalarE
exp stream (the steady-state bottleneck) starts early and the remaining
projections hide under it.
"""

import sys
import types

sys.path.insert(0, "/opt/trn_rl_repo")

import numpy as np

# Best-effort: register the axon NTFF profile hook so trace=True works
# (used by test harnesses via BASS_TRACE); harmless when tracing is off.
try:
    import antenv

    if "antenv.axon_hooks" not in sys.modules:
        from trn_agent_boot.trn_boot import _ntff_profile_via_ctypes

        _hooks_mod = types.ModuleType("antenv.axon_hooks")
        _hook = _ntff_profile_via_ctypes("/opt/axon/libaxon_pjrt.so")
        _hooks_mod.get_axon_ntff_profile_hook = lambda: _hook
        _hooks_mod.set_axon_ntff_profile_hook = lambda h: None
        sys.modules["antenv.axon_hooks"] = _hooks_mod
        antenv.axon_hooks = _hooks_mod
except Exception:
    pass

import concourse.bacc as bacc
import concourse.tile as tile
from concourse import mybir
from concourse.bass_utils import run_bass_kernel_spmd

B, N, F = 4, 2048, 1024
HEAD, HD = 16, 64
NCORES = 8
HG = 2                # head groups (tensor-parallel degree per batch)
FL = F // HG          # local features per core = 512
HL = HEAD // HG       # local heads per core = 8
OC = FL // 128        # o-chunks of 128 in qT/kT = 4
FT = F // 128         # f (contraction) tiles = 8
IC = N // 128         # i/j chunks of 128 = 16
ISPAN = 1024          # attention i-span per inner block
NSP = N // ISPAN      # spans = 2

BF16 = mybir.dt.bfloat16
F32 = mybir.dt.float32
NP_BF16 = mybir.dt.np(BF16)

_CACHED_NC = None
LAST_EXEC_TIME_NS = None
LAST_RES = None


def _build_nc():
    nc = bacc.Bacc("TRN2")

    xT = nc.dram_tensor("xT", [F, N], BF16, kind="ExternalInput")
    wqT = nc.dram_tensor("wqT", [F, FL], BF16, kind="ExternalInput")
    wkT = nc.dram_tensor("wkT", [F, FL], BF16, kind="ExternalInput")
    wvT = nc.dram_tensor("wvT", [F, FL], BF16, kind="ExternalInput")
    woT = nc.dram_tensor("woT", [FL, F], BF16, kind="ExternalInput")
    bq = nc.dram_tensor("bq", [FL], F32, kind="ExternalInput")
    bk = nc.dram_tensor("bk", [FL], F32, kind="ExternalInput")
    bv = nc.dram_tensor("bv", [FL], F32, kind="ExternalInput")
    boh = nc.dram_tensor("boh", [F], F32, kind="ExternalInput")
    out = nc.dram_tensor("out", [N, F], F32, kind="ExternalOutput")

    with tile.TileContext(nc) as tc:
        with (
            tc.tile_pool(name="big", bufs=1) as big,
            tc.tile_pool(name="etile", bufs=4) as epool,
            tc.tile_pool(name="auc", bufs=3) as aucp,
            tc.tile_pool(name="ztile", bufs=3) as zpool,
            tc.tile_pool(name="rbc", bufs=3) as rpool,
            tc.tile_pool(name="ostage", bufs=3) as opool,
            tc.tile_pool(name="dspill", bufs=4, space="DRAM") as dpool,
            tc.tile_pool(name="pp", bufs=2, space="PSUM") as pp,
            tc.tile_pool(name="sp", bufs=2, space="PSUM") as spp,
            tc.tile_pool(name="aup", bufs=1, space="PSUM") as aup,
        ):
            # ---- resident SBUF tensors ----
            xT_sb = big.tile([128, FT, N], BF16, tag="xT")
            wqT_sb = big.tile([128, FT, FL], BF16, tag="wqT")
            wkT_sb = big.tile([128, FT, FL], BF16, tag="wkT")
            wvT_sb = big.tile([128, FT, FL], BF16, tag="wvT")
            woT_sb = big.tile([128, OC, F], BF16, tag="woT")
            qT_oc = [big.tile([128, N], BF16, tag=f"qT{oc}", name=f"qT{oc}") for oc in range(OC)]
            kT_oc = [big.tile([128, N], BF16, tag=f"kT{oc}", name=f"kT{oc}") for oc in range(OC)]
            vAug_sb = big.tile([128, IC, HL, HD + 1], BF16, tag="vAug")
            attnT_oc = [big.tile([128, N], BF16, tag=f"attnT{oc}", name=f"attnT{oc}") for oc in range(OC)]
            bq_sb = big.tile([128, OC], F32, tag="bq")
            bk_sb = big.tile([128, OC], F32, tag="bk")
            bv_sb = big.tile([128, FL], F32, tag="bv")
            bo_sb = big.tile([128, F], F32, tag="bo")

            # ---- input DMAs ----
            nc.sync.dma_start(
                out=xT_sb[:], in_=xT.rearrange("(t p) i -> p t i", p=128)
            )
            nc.sync.dma_start(
                out=wqT_sb[:], in_=wqT.rearrange("(t p) o -> p t o", p=128)
            )
            nc.sync.dma_start(
                out=wkT_sb[:], in_=wkT.rearrange("(t p) o -> p t o", p=128)
            )
            nc.sync.dma_start(
                out=wvT_sb[:], in_=wvT.rearrange("(t p) o -> p t o", p=128)
            )
            nc.sync.dma_start(
                out=woT_sb[:], in_=woT.rearrange("(t p) g -> p t g", p=128)
            )
            nc.sync.dma_start(
                out=bq_sb[:], in_=bq.rearrange("(c p) -> p c", p=128)
            )
            nc.sync.dma_start(
                out=bk_sb[:], in_=bk.rearrange("(c p) -> p c", p=128)
            )
            nc.sync.dma_start(out=bv_sb[:], in_=bv[None, :].partition_broadcast(128))
            nc.sync.dma_start(out=bo_sb[:], in_=boh[None, :].partition_broadcast(128))
            # ones column (64) for every head; V overwrites columns 0..63
            nc.vector.memset(vAug_sb[:], 1.0)

            # ---- v projection first (attention needs all of vAug) ----
            for ic in range(IC):
                ps = pp.tile([128, 512], F32, tag="pp")
                for t in range(FT):
                    nc.tensor.matmul(
                        ps[:],
                        lhsT=xT_sb[:, t, ic * 128 : (ic + 1) * 128],
                        rhs=wvT_sb[:, t, :],
                        start=(t == 0),
                        stop=(t == FT - 1),
                    )
                nc.vector.tensor_add(
                    out=vAug_sb[:, ic, :, 0:HD],
                    in0=ps.rearrange("p (h d) -> p h d", h=HL),
                    in1=bv_sb.rearrange("p (h d) -> p h d", h=HL),
                )

            def project_qk(w_sb, b_sb, dst, oc):
                for ic in range(N // 512):
                    ps = pp.tile([128, 512], F32, tag="pp")
                    for t in range(FT):
                        nc.tensor.matmul(
                            ps[:],
                            lhsT=w_sb[:, t, oc * 128 : (oc + 1) * 128],
                            rhs=xT_sb[:, t, ic * 512 : (ic + 1) * 512],
                            start=(t == 0),
                            stop=(t == FT - 1),
                        )
                    nc.vector.tensor_scalar_add(
                        out=dst[:, ic * 512 : (ic + 1) * 512],
                        in0=ps[:],
                        scalar1=b_sb[:, oc : oc + 1],
                    )

            def attention_head(h):
                oc, sub = h // 2, h % 2
                p0 = sub * 64
                qh = qT_oc[oc][p0 : p0 + 64, :]
                kh = kT_oc[oc][p0 : p0 + 64, :]
                for isp in range(NSP):
                    i0 = isp * ISPAN
                    au = aup.tile([HD + 1, ISPAN], F32, tag="au")
                    for j in range(IC):
                        st = spp.tile([128, ISPAN], F32, tag="sp")
                        for ii in range(ISPAN // 512):
                            nc.tensor.matmul(
                                st[:, ii * 512 : (ii + 1) * 512],
                                lhsT=kh[:, j * 128 : (j + 1) * 128],
                                rhs=qh[:, i0 + ii * 512 : i0 + (ii + 1) * 512],
                                start=True,
                                stop=True,
                            )
                        eT = epool.tile([128, ISPAN], BF16, tag="eT")
                        nc.scalar.activation(
                            eT[:], st[:], mybir.ActivationFunctionType.Exp,
                            scale=1.0 / 32.0,
                        )
                        for ii in range(ISPAN // 512):
                            nc.tensor.matmul(
                                au[:, ii * 512 : (ii + 1) * 512],
                                lhsT=vAug_sb[:, j, h, :],
                                rhs=eT[:, ii * 512 : (ii + 1) * 512],
                                start=(j == 0),
                                stop=(j == IC - 1),
                            )
                    # copy attnU + Z out of PSUM promptly to release the bank
                    auc = aucp.tile([HD + 1, ISPAN], F32, tag="auc")
                    nc.vector.tensor_copy(auc[:], au[:])
                    # 1/Z with decent parallelism: bounce Z through DRAM into a
                    # [128, ISPAN/128] layout, reciprocal, bounce back broadcast
                    zd = dpool.tile([1, ISPAN], F32, tag="zd")
                    nc.sync.dma_start(out=zd[:], in_=auc[HD : HD + 1, :])
                    zs = zpool.tile([128, ISPAN // 128], F32, tag="zs")
                    nc.sync.dma_start(
                        out=zs[:], in_=zd[0, :].rearrange("(p f) -> p f", p=128)
                    )
                    zr = zpool.tile([128, ISPAN // 128], F32, tag="zr")
                    nc.vector.reciprocal(zr[:], zs[:])
                    zrd = dpool.tile([1, ISPAN], F32, tag="zrd")
                    nc.sync.dma_start(
                        out=zrd[0, :].rearrange("(p f) -> p f", p=128), in_=zr[:]
                    )
                    rb = rpool.tile([64, ISPAN], F32, tag="rb")
                    nc.sync.dma_start(
                        out=rb[:], in_=zrd[0, :].partition_broadcast(64)
                    )
                    nc.vector.tensor_mul(
                        out=attnT_oc[oc][p0 : p0 + 64, i0 : i0 + ISPAN],
                        in0=auc[0:HD, :],
                        in1=rb[:],
                    )

            # ---- interleaved q/k projection + attention ----
            for oc in range(OC):
                project_qk(wqT_sb, bq_sb, qT_oc[oc], oc)
                project_qk(wkT_sb, bk_sb, kT_oc[oc], oc)
                attention_head(2 * oc)
                attention_head(2 * oc + 1)

            # ---- o projection (partial: local 512 features only) ----
            for ic in range(IC):
                for gc in range(F // 512):
                    ps = pp.tile([128, 512], F32, tag="pp")
                    for ct in range(OC):
                        nc.tensor.matmul(
                            ps[:],
                            lhsT=attnT_oc[ct][:, ic * 128 : (ic + 1) * 128],
                            rhs=woT_sb[:, ct, gc * 512 : (gc + 1) * 512],
                            start=(ct == 0),
                            stop=(ct == OC - 1),
                        )
                    st = opool.tile([128, 512], F32, tag="ost")
                    nc.vector.tensor_add(
                        out=st[:], in0=ps[:], in1=bo_sb[:, gc * 512 : (gc + 1) * 512]
                    )
                    nc.sync.dma_start(
                        out=out[ic * 128 : (ic + 1) * 128, gc * 512 : (gc + 1) * 512],
                        in_=st[:],
                    )

    nc.finalize()
    return nc


def kernel(x, Wq, bq, Wk, bk, Wv, bv, Wo, bo, trace=False):
    global _CACHED_NC, LAST_EXEC_TIME_NS, LAST_RES
    x = np.asarray(x)
    Wq, Wk, Wv, Wo = (np.asarray(a) for a in (Wq, Wk, Wv, Wo))
    bq, bk, bv, bo = (np.asarray(a) for a in (bq, bk, bv, bo))

    if _CACHED_NC is None:
        _CACHED_NC = _build_nc()
    nc = _CACHED_NC

    # host-side shard prep (transposes + bf16 casts)
    xT_b = [np.ascontiguousarray(x[b].T).astype(NP_BF16) for b in range(B)]
    wqT_g = [np.ascontiguousarray(Wq[g * FL : (g + 1) * FL, :].T).astype(NP_BF16) for g in range(HG)]
    wkT_g = [np.ascontiguousarray(Wk[g * FL : (g + 1) * FL, :].T).astype(NP_BF16) for g in range(HG)]
    wvT_g = [np.ascontiguousarray(Wv[g * FL : (g + 1) * FL, :].T).astype(NP_BF16) for g in range(HG)]
    woT_g = [np.ascontiguousarray(Wo[:, g * FL : (g + 1) * FL].T).astype(NP_BF16) for g in range(HG)]
    bq_g = [np.ascontiguousarray(bq[g * FL : (g + 1) * FL]).astype(np.float32) for g in range(HG)]
    bk_g = [np.ascontiguousarray(bk[g * FL : (g + 1) * FL]).astype(np.float32) for g in range(HG)]
    bv_g = [np.ascontiguousarray(bv[g * FL : (g + 1) * FL]).astype(np.float32) for g in range(HG)]
    bo_half = (bo.astype(np.float32) / 2.0)

    in_maps = []
    for c in range(NCORES):
        b, g = c // HG, c % HG
        in_maps.append(
            {
                "xT": xT_b[b],
                "wqT": wqT_g[g],
                "wkT": wkT_g[g],
                "wvT": wvT_g[g],
                "woT": woT_g[g],
                "bq": bq_g[g],
                "bk": bk_g[g],
                "bv": bv_g[g],
                "boh": bo_half,
            }
        )

    res = run_bass_kernel_spmd(nc, in_maps, core_ids=list(range(NCORES)), trace=trace)
    LAST_EXEC_TIME_NS = res.exec_time_ns
    LAST_RES = res

    out = np.empty((B, N, F), np.float32)
    for b in range(B):
        out[b] = res.results[2 * b]["out"] + res.results[2 * b + 1]["out"]
    return out


# revision 7
# speedup vs baseline: 1.6926x; 1.1355x over previous
"""Multi-head self-attention (B=4, N=2048, F=1024, 16 heads) on 8 TRN2 NeuronCores.

Sharding (Megatron-style, per the hint): data-parallel over the 4 batches x
tensor-parallel split of the 16 heads into 2 groups of 8. Core c handles
batch c//2 with head group c%2 (512 of the 1024 qkv features, column-split
Wq/Wk/Wv, row-split Wo). Each core emits a partial o-projection [2048, 1024];
the host unshard sums the pair of partials per batch (the Megatron
all-reduce) and stacks batches.

Device kernel layout choices (all matmuls bf16 with f32 PSUM accumulation):
  xT   [1024f, 2048i]  (x transposed on host)  - rhs for q/k, lhsT for v
  qT/kT [512o, 2048i]  (o = head-major features, on partitions)
  vAug [2048j, 8h, 65] (per head: V columns 0..63 plus a ones column at 64
                        so the attention-value matmul also yields the softmax
                        denominator Z as output row 64)
  scores S^T [j, i] via lhsT=kT-chunk, rhs=qT; exp on ScalarE (scale=1/32,
  no max subtraction needed: |S/32| <~ 1.5); attnU^T accumulated over j in
  PSUM, copied to SBUF promptly to release the PSUM bank; softmax
  normalization (1/Z broadcast) runs off the critical path via small DMA
  round-trips through DRAM.

QKV is interleaved with attention per 128-row chunk of q/k so the ScalarE
exp stream (the steady-state bottleneck) starts early and the remaining
projections hide under it.
"""

import sys
import types

sys.path.insert(0, "/opt/trn_rl_repo")

import numpy as np

# Best-effort: register the axon NTFF profile hook so trace=True works
# (used by test harnesses via BASS_TRACE); harmless when tracing is off.
try:
    import antenv

    if "antenv.axon_hooks" not in sys.modules:
        from trn_agent_boot.trn_boot import _ntff_profile_via_ctypes

        _hooks_mod = types.ModuleType("antenv.axon_hooks")
        _hook = _ntff_profile_via_ctypes("/opt/axon/libaxon_pjrt.so")
        _hooks_mod.get_axon_ntff_profile_hook = lambda: _hook
        _hooks_mod.set_axon_ntff_profile_hook = lambda h: None
        sys.modules["antenv.axon_hooks"] = _hooks_mod
        antenv.axon_hooks = _hooks_mod
except Exception:
    pass

import concourse.bacc as bacc
import concourse.tile as tile
from concourse import mybir
from concourse.bass_utils import run_bass_kernel_spmd

B, N, F = 4, 2048, 1024
HEAD, HD = 16, 64
NCORES = 8
HG = 2                # head groups (tensor-parallel degree per batch)
FL = F // HG          # local features per core = 512
HL = HEAD // HG       # local heads per core = 8
OC = FL // 128        # o-chunks of 128 in qT/kT = 4
FT = F // 128         # f (contraction) tiles = 8
IC = N // 128         # i/j chunks of 128 = 16
ISPAN = 1024          # attention i-span per inner block
NSP = N // ISPAN      # spans = 2

BF16 = mybir.dt.bfloat16
F32 = mybir.dt.float32
NP_BF16 = mybir.dt.np(BF16)

_CACHED_NC = None
LAST_EXEC_TIME_NS = None
LAST_RES = None


def _build_nc():
    nc = bacc.Bacc("TRN2")

    xT = nc.dram_tensor("xT", [F, N], BF16, kind="ExternalInput")
    wqT = nc.dram_tensor("wqT", [F, FL], BF16, kind="ExternalInput")
    wkT = nc.dram_tensor("wkT", [F, FL], BF16, kind="ExternalInput")
    wvT = nc.dram_tensor("wvT", [F, FL], BF16, kind="ExternalInput")
    woT = nc.dram_tensor("woT", [FL, F], BF16, kind="ExternalInput")
    bq = nc.dram_tensor("bq", [FL], F32, kind="ExternalInput")
    bk = nc.dram_tensor("bk", [FL], F32, kind="ExternalInput")
    bv = nc.dram_tensor("bv", [FL], F32, kind="ExternalInput")
    boh = nc.dram_tensor("boh", [F], F32, kind="ExternalInput")
    out = nc.dram_tensor("out", [N, F], F32, kind="ExternalOutput")

    with tile.TileContext(nc) as tc:
        with (
            tc.tile_pool(name="big", bufs=1) as big,
            tc.tile_pool(name="etile", bufs=4) as epool,
            tc.tile_pool(name="auc", bufs=3) as aucp,
            tc.tile_pool(name="ztile", bufs=3) as zpool,
            tc.tile_pool(name="rbc", bufs=3) as rpool,
            tc.tile_pool(name="ostage", bufs=3) as opool,
            tc.tile_pool(name="dspill", bufs=4, space="DRAM") as dpool,
            tc.tile_pool(name="pp", bufs=2, space="PSUM") as pp,
            tc.tile_pool(name="sp", bufs=2, space="PSUM") as spp,
            tc.tile_pool(name="aup", bufs=1, space="PSUM") as aup,
        ):
            # ---- resident SBUF tensors ----
            xT_sb = big.tile([128, FT, N], BF16, tag="xT")
            wqT_sb = big.tile([128, FT, FL], BF16, tag="wqT")
            wkT_sb = big.tile([128, FT, FL], BF16, tag="wkT")
            wvT_sb = big.tile([128, FT, FL], BF16, tag="wvT")
            woT_sb = big.tile([128, OC, F], BF16, tag="woT")
            qT_oc = [big.tile([128, N], BF16, tag=f"qT{oc}", name=f"qT{oc}") for oc in range(OC)]
            kT_oc = [big.tile([128, N], BF16, tag=f"kT{oc}", name=f"kT{oc}") for oc in range(OC)]
            vAug_sb = big.tile([128, IC, HL, HD + 1], BF16, tag="vAug")
            attnT_oc = [big.tile([128, N], BF16, tag=f"attnT{oc}", name=f"attnT{oc}") for oc in range(OC)]
            bq_sb = big.tile([128, OC], F32, tag="bq")
            bk_sb = big.tile([128, OC], F32, tag="bk")
            bv_sb = big.tile([128, FL], F32, tag="bv")
            bo_sb = big.tile([128, F], F32, tag="bo")

            # ---- input DMAs ----
            nc.sync.dma_start(
                out=xT_sb[:], in_=xT.rearrange("(t p) i -> p t i", p=128)
            )
            nc.sync.dma_start(
                out=wqT_sb[:], in_=wqT.rearrange("(t p) o -> p t o", p=128)
            )
            nc.sync.dma_start(
                out=wkT_sb[:], in_=wkT.rearrange("(t p) o -> p t o", p=128)
            )
            nc.sync.dma_start(
                out=wvT_sb[:], in_=wvT.rearrange("(t p) o -> p t o", p=128)
            )
            nc.sync.dma_start(
                out=woT_sb[:], in_=woT.rearrange("(t p) g -> p t g", p=128)
            )
            nc.sync.dma_start(
                out=bq_sb[:], in_=bq.rearrange("(c p) -> p c", p=128)
            )
            nc.sync.dma_start(
                out=bk_sb[:], in_=bk.rearrange("(c p) -> p c", p=128)
            )
            nc.sync.dma_start(out=bv_sb[:], in_=bv[None, :].partition_broadcast(128))
            nc.sync.dma_start(out=bo_sb[:], in_=boh[None, :].partition_broadcast(128))
            # ones column (64) for every head; V overwrites columns 0..63
            nc.vector.memset(vAug_sb[:], 1.0)

            # ---- v projection first (attention needs all of vAug) ----
            for ic in range(IC):
                ps = pp.tile([128, 512], F32, tag="pp")
                for t in range(FT):
                    nc.tensor.matmul(
                        ps[:],
                        lhsT=xT_sb[:, t, ic * 128 : (ic + 1) * 128],
                        rhs=wvT_sb[:, t, :],
                        start=(t == 0),
                        stop=(t == FT - 1),
                    )
                nc.vector.tensor_add(
                    out=vAug_sb[:, ic, :, 0:HD],
                    in0=ps.rearrange("p (h d) -> p h d", h=HL),
                    in1=bv_sb.rearrange("p (h d) -> p h d", h=HL),
                )

            def project_qk(w_sb, b_sb, dst, oc):
                for ic in range(N // 512):
                    ps = pp.tile([128, 512], F32, tag="pp")
                    for t in range(FT):
                        nc.tensor.matmul(
                            ps[:],
                            lhsT=w_sb[:, t, oc * 128 : (oc + 1) * 128],
                            rhs=xT_sb[:, t, ic * 512 : (ic + 1) * 512],
                            start=(t == 0),
                            stop=(t == FT - 1),
                        )
                    nc.vector.tensor_scalar_add(
                        out=dst[:, ic * 512 : (ic + 1) * 512],
                        in0=ps[:],
                        scalar1=b_sb[:, oc : oc + 1],
                    )

            def attention_pair(oc):
                # heads h0 = 2*oc (q/k rows 0:64) and h1 = 2*oc+1 (rows 64:128)
                # are processed together: their score matmuls sit on disjoint
                # PE row-groups (K=64 at base partition 0 vs 64) and run
                # concurrently; one [128, 1024] S-PSUM tile holds a 512-wide
                # i-span for each head so exp still works in [128,1024] calls.
                h0, h1 = 2 * oc, 2 * oc + 1
                q0 = qT_oc[oc][0:64, :]
                k0 = kT_oc[oc][0:64, :]
                q1 = qT_oc[oc][64:128, :]
                k1 = kT_oc[oc][64:128, :]
                for isp in range(N // 512):
                    i0 = isp * 512
                    au = aup.tile([HD + 1, 1024], F32, tag="au")
                    for j in range(IC):
                        st = spp.tile([128, 1024], F32, tag="sp")
                        nc.tensor.matmul(
                            st[:, 0:512],
                            lhsT=k0[:, j * 128 : (j + 1) * 128],
                            rhs=q0[:, i0 : i0 + 512],
                            start=True,
                            stop=True,
                        )
                        nc.tensor.matmul(
                            st[:, 512:1024],
                            lhsT=k1[:, j * 128 : (j + 1) * 128],
                            rhs=q1[:, i0 : i0 + 512],
                            start=True,
                            stop=True,
                        )
                        eT = epool.tile([128, 1024], BF16, tag="eT")
                        nc.scalar.activation(
                            eT[:], st[:], mybir.ActivationFunctionType.Exp,
                            scale=1.0 / 32.0,
                        )
                        nc.tensor.matmul(
                            au[:, 0:512],
                            lhsT=vAug_sb[:, j, h0, :],
                            rhs=eT[:, 0:512],
                            start=(j == 0),
                            stop=(j == IC - 1),
                        )
                        nc.tensor.matmul(
                            au[:, 512:1024],
                            lhsT=vAug_sb[:, j, h1, :],
                            rhs=eT[:, 512:1024],
                            start=(j == 0),
                            stop=(j == IC - 1),
                        )
                    # copy attnU + Z out of PSUM promptly to release the bank
                    auc = aucp.tile([HD + 1, 1024], F32, tag="auc")
                    nc.vector.tensor_copy(auc[:], au[:])
                    # 1/Z with decent parallelism: bounce Z through DRAM into a
                    # [128, 8] layout, reciprocal, bounce back broadcast
                    zd = dpool.tile([1, 1024], F32, tag="zd")
                    nc.sync.dma_start(out=zd[:], in_=auc[HD : HD + 1, :])
                    zs = zpool.tile([128, 8], F32, tag="zs")
                    nc.sync.dma_start(
                        out=zs[:], in_=zd[0, :].rearrange("(p f) -> p f", p=128)
                    )
                    zr = zpool.tile([128, 8], F32, tag="zr")
                    nc.vector.reciprocal(zr[:], zs[:])
                    zrd = dpool.tile([1, 1024], F32, tag="zrd")
                    nc.sync.dma_start(
                        out=zrd[0, :].rearrange("(p f) -> p f", p=128), in_=zr[:]
                    )
                    rb = rpool.tile([64, 1024], F32, tag="rb")
                    nc.sync.dma_start(
                        out=rb[:], in_=zrd[0, :].partition_broadcast(64)
                    )
                    nc.vector.tensor_mul(
                        out=attnT_oc[oc][0:64, i0 : i0 + 512],
                        in0=auc[0:HD, 0:512],
                        in1=rb[:, 0:512],
                    )
                    nc.vector.tensor_mul(
                        out=attnT_oc[oc][64:128, i0 : i0 + 512],
                        in0=auc[0:HD, 512:1024],
                        in1=rb[:, 512:1024],
                    )

            # ---- interleaved q/k projection + attention ----
            for oc in range(OC):
                project_qk(wqT_sb, bq_sb, qT_oc[oc], oc)
                project_qk(wkT_sb, bk_sb, kT_oc[oc], oc)
                attention_pair(oc)

            # ---- o projection (partial: local 512 features only) ----
            for ic in range(IC):
                for gc in range(F // 512):
                    ps = pp.tile([128, 512], F32, tag="pp")
                    for ct in range(OC):
                        nc.tensor.matmul(
                            ps[:],
                            lhsT=attnT_oc[ct][:, ic * 128 : (ic + 1) * 128],
                            rhs=woT_sb[:, ct, gc * 512 : (gc + 1) * 512],
                            start=(ct == 0),
                            stop=(ct == OC - 1),
                        )
                    st = opool.tile([128, 512], F32, tag="ost")
                    nc.vector.tensor_add(
                        out=st[:], in0=ps[:], in1=bo_sb[:, gc * 512 : (gc + 1) * 512]
                    )
                    nc.sync.dma_start(
                        out=out[ic * 128 : (ic + 1) * 128, gc * 512 : (gc + 1) * 512],
                        in_=st[:],
                    )

    nc.finalize()
    return nc


def kernel(x, Wq, bq, Wk, bk, Wv, bv, Wo, bo, trace=False):
    global _CACHED_NC, LAST_EXEC_TIME_NS, LAST_RES
    x = np.asarray(x)
    Wq, Wk, Wv, Wo = (np.asarray(a) for a in (Wq, Wk, Wv, Wo))
    bq, bk, bv, bo = (np.asarray(a) for a in (bq, bk, bv, bo))

    if _CACHED_NC is None:
        _CACHED_NC = _build_nc()
    nc = _CACHED_NC

    # host-side shard prep (transposes + bf16 casts)
    xT_b = [np.ascontiguousarray(x[b].T).astype(NP_BF16) for b in range(B)]
    wqT_g = [np.ascontiguousarray(Wq[g * FL : (g + 1) * FL, :].T).astype(NP_BF16) for g in range(HG)]
    wkT_g = [np.ascontiguousarray(Wk[g * FL : (g + 1) * FL, :].T).astype(NP_BF16) for g in range(HG)]
    wvT_g = [np.ascontiguousarray(Wv[g * FL : (g + 1) * FL, :].T).astype(NP_BF16) for g in range(HG)]
    woT_g = [np.ascontiguousarray(Wo[:, g * FL : (g + 1) * FL].T).astype(NP_BF16) for g in range(HG)]
    bq_g = [np.ascontiguousarray(bq[g * FL : (g + 1) * FL]).astype(np.float32) for g in range(HG)]
    bk_g = [np.ascontiguousarray(bk[g * FL : (g + 1) * FL]).astype(np.float32) for g in range(HG)]
    bv_g = [np.ascontiguousarray(bv[g * FL : (g + 1) * FL]).astype(np.float32) for g in range(HG)]
    bo_half = (bo.astype(np.float32) / 2.0)

    in_maps = []
    for c in range(NCORES):
        b, g = c // HG, c % HG
        in_maps.append(
            {
                "xT": xT_b[b],
                "wqT": wqT_g[g],
                "wkT": wkT_g[g],
                "wvT": wvT_g[g],
                "woT": woT_g[g],
                "bq": bq_g[g],
                "bk": bk_g[g],
                "bv": bv_g[g],
                "boh": bo_half,
            }
        )

    res = run_bass_kernel_spmd(nc, in_maps, core_ids=list(range(NCORES)), trace=trace)
    LAST_EXEC_TIME_NS = res.exec_time_ns
    LAST_RES = res

    out = np.empty((B, N, F), np.float32)
    for b in range(B):
        out[b] = res.results[2 * b]["out"] + res.results[2 * b + 1]["out"]
    return out
